# revision 15
# baseline (speedup 1.0000x reference)
"""Chamfer distance loss kernel for Trainium2 (8 NeuronCores).

Problem: template/source [B=4, N=8192, D=3] fp32.
  D[b,n,m] = ||template[b,n] - source[b,m]||
  out = mean_bm(min_n D) + mean_bn(min_m D)

Strategy:
  - sqrt is monotonic: min(sqrt(x)) = sqrt(min(x)); clamp commutes with min.
  - Squared-distance row mins via augmented matmul:
      E[n,m] = b2[m] - 2*a[n]@b[m] = lhsT.T @ rhs with
      lhsT = [-2a_x; -2a_y; -2a_z; 1] (K x Nchunk), rhs = [b_x; b_y; b_z; b2]
    then rowmin_n = relu(a2[n] + min_m E[n,m]); dist = sqrt.
  - Both min directions are computed independently (swap roles of a/b), so
    every reduction is a free-axis (DVE) reduction -- no partition reductions.
  - Sharding: core c handles batch b=c//2, half h=c%2 (4096 rows) for BOTH
    directions: rows of the template chunk vs all sources (row-dir), and rows
    of the source chunk vs all templates (col-dir). No cross-core comms.
  - Device returns per-partition sums of sqrt'd mins [128, 2]; host sums and
    divides.
"""

import os
import numpy as np
from contextlib import ExitStack

import concourse.bass as bass
import concourse.tile as tile
from concourse import mybir
from concourse.bass_utils import run_bass_kernel_spmd

B, N, D = 4, 8192, 3
M = N
N_CORES = 8
HALF = N // 2            # 4096 rows per core per direction
NBLK = HALF // 128       # 32 row blocks of 128
MTILE = 512              # matmul moving free dim (one PSUM bank)
MBLK = M // MTILE        # 16 column tiles
KAUG = 4                 # [-2a_x, -2a_y, -2a_z, 1]
KSPL = 4 * D + 2         # v2: 2-term f32r split: 4 pairs/dim + 2 rows for b2
GRP = 4                  # v2: j-tiles fused per TTR group (2 psum kilotiles)

F32 = mybir.dt.float32
F32R = mybir.dt.float32r

MODE = os.environ.get("CHAMFER_MODE", "v2")

def _split_excess_waits(nc, limit=1):
    """Walrus rejects instructions with more than `limit` sem waits (seen for
    both CTRL drains and matmul LDW encodings). Hoist the excess onto
    standalone InstEventSemaphore instructions inserted right before, on the
    same engine (same-engine program order makes this semantically
    identical)."""
    f = nc.m.functions[0]
    n_split = 0
    for bb in f.blocks:
        new_list = []
        for inst in bb.instructions:
            si = inst.sync_info
            waits = list(si.on_wait) if si and si.on_wait else []
            if len(waits) > limit:
                keep, excess = waits[:limit], waits[limit:]
                for k, w in enumerate(excess):
                    ev = mybir.InstEventSemaphore(
                        name=f"splitw_{inst.name}_{k}", ins=[], outs=[]
                    )
                    ev.engine = inst.engine
                    ev.sync_info = mybir.SyncInfo(on_wait=[w], on_update=[])
                    nc.register_instruction(ev)
                    new_list.append(ev)
                    n_split += 1
                inst.sync_info = mybir.SyncInfo(
                    on_wait=keep,
                    on_update=list(si.on_update) if si.on_update else [],
                )
            new_list.append(inst)
        bb.instructions[:] = new_list
    return n_split


def build_kernel(reps=1, mode=None):
    mode = mode or MODE
    if mode == "v2":
        return build_kernel_v2(reps)
    nc = bass.Bass(
        trn_type="TRN2", target_bir_lowering=False, debug=False,
        num_devices=N_CORES,
    )
    lhs_row = nc.dram_tensor("lhs_row", [KAUG, HALF], F32, kind="ExternalInput").ap()
    rhs_row = nc.dram_tensor("rhs_row", [KAUG, M], F32, kind="ExternalInput").ap()
    lhs_col = nc.dram_tensor("lhs_col", [KAUG, HALF], F32, kind="ExternalInput").ap()
    rhs_col = nc.dram_tensor("rhs_col", [KAUG, N], F32, kind="ExternalInput").ap()
    add_row = nc.dram_tensor("add_row", [128, NBLK], F32, kind="ExternalInput").ap()
    add_col = nc.dram_tensor("add_col", [128, NBLK], F32, kind="ExternalInput").ap()
    out = nc.dram_tensor("out", [128, 2], F32, kind="ExternalOutput").ap()

    with tile.TileContext(nc) as tc, ExitStack() as ctx:
        const_pool = ctx.enter_context(tc.tile_pool(name="const", bufs=1))
        psum_pool = ctx.enter_context(tc.tile_pool(name="psum", bufs=6, space="PSUM"))
        work_pool = ctx.enter_context(tc.tile_pool(name="work", bufs=3))

        lr = const_pool.tile([KAUG, HALF], F32, tag="lr")
        nc.gpsimd.dma_start(lr[:], lhs_row[:])
        rr = const_pool.tile([KAUG, M], F32, tag="rr")
        nc.gpsimd.dma_start(rr[:], rhs_row[:])
        lc = const_pool.tile([KAUG, HALF], F32, tag="lc")
        nc.gpsimd.dma_start(lc[:], lhs_col[:])
        rc = const_pool.tile([KAUG, N], F32, tag="rc")
        nc.gpsimd.dma_start(rc[:], rhs_col[:])
        ar = const_pool.tile([128, NBLK], F32, tag="ar")
        nc.gpsimd.dma_start(ar[:], add_row[:])
        ac = const_pool.tile([128, NBLK], F32, tag="ac")
        nc.gpsimd.dma_start(ac[:], add_col[:])

        def body(_iv=None):
            sums = work_pool.tile([128, 2], F32, tag="sums")
            for d, (lhs_s, rhs_s, add_s) in enumerate(
                [(lr, rr, ar), (lc, rc, ac)]
            ):
                mins_all = work_pool.tile([128, NBLK], F32, tag="mins_all")
                for i in range(NBLK):
                    minbuf = work_pool.tile([128, MBLK], F32, tag="minbuf")
                    for j in range(MBLK):
                        pt = psum_pool.tile([128, MTILE], F32, tag="pt")
                        nc.tensor.matmul(
                            out=pt[:],
                            lhsT=lhs_s[:, i * 128:(i + 1) * 128],
                            rhs=rhs_s[:, j * MTILE:(j + 1) * MTILE],
                            start=True, stop=True,
                        )
                        nc.vector.tensor_reduce(
                            out=minbuf[:, j:j + 1], in_=pt[:],
                            axis=mybir.AxisListType.X, op=mybir.AluOpType.min,
                        )
                    nc.vector.tensor_reduce(
                        out=mins_all[:, i:i + 1], in_=minbuf[:],
                        axis=mybir.AxisListType.X, op=mybir.AluOpType.min,
                    )
                # sq = relu(mins + a2); dist = sqrt(sq); per-partition sum
                sq = work_pool.tile([128, NBLK], F32, tag="sq")
                nc.vector.tensor_tensor(
                    out=sq[:], in0=mins_all[:], in1=add_s[:],
                    op=mybir.AluOpType.add,
                )
                nc.scalar.activation(
                    out=sq[:], in_=sq[:], func=mybir.ActivationFunctionType.Relu,
                )
                nc.scalar.activation(
                    out=sq[:], in_=sq[:], func=mybir.ActivationFunctionType.Sqrt,
                )
                nc.vector.tensor_reduce(
                    out=sums[:, d:d + 1], in_=sq[:],
                    axis=mybir.AxisListType.X, op=mybir.AluOpType.add,
                )
            nc.gpsimd.dma_start(out[:], sums[:])

        for _ in range(reps):
            body()

    _split_excess_waits(nc)
    return nc


def build_kernel_v2(reps=1):
    """f32r exact-split matmuls (K=14, 1 cyc/row) + wide DVE min-reduces.

    Per 128-row strip: PE fills [128,2048] PSUM kilotiles (4 matmuls each,
    4 banks); DVE tensor_reduce(min) consumes each kilotile directly from
    PSUM (1 elem/cycle — the fused tensor_tensor_reduce and all 2-input
    GPSIMD ops are rejected by this walrus build, so DVE examination rate
    is the floor). PSUM double-buffered: 2 kilotiles = all 8 banks."""
    nc = bass.Bass(
        trn_type="TRN2", target_bir_lowering=False, debug=False,
        num_devices=N_CORES,
    )
    lhs_row = nc.dram_tensor("lhs_row", [KSPL, HALF], F32R, kind="ExternalInput").ap()
    rhs_row = nc.dram_tensor("rhs_row", [KSPL, M], F32R, kind="ExternalInput").ap()
    lhs_col = nc.dram_tensor("lhs_col", [KSPL, HALF], F32R, kind="ExternalInput").ap()
    rhs_col = nc.dram_tensor("rhs_col", [KSPL, N], F32R, kind="ExternalInput").ap()
    add_row = nc.dram_tensor("add_row", [128, NBLK], F32, kind="ExternalInput").ap()
    add_col = nc.dram_tensor("add_col", [128, NBLK], F32, kind="ExternalInput").ap()
    out = nc.dram_tensor("out", [128, 2], F32, kind="ExternalOutput").ap()

    W = 2048                  # kilotile width: 4 matmul tiles, 4 PSUM banks
    NK = MBLK * MTILE // W    # 4 kilotiles per row strip

    with tile.TileContext(nc) as tc, ExitStack() as ctx:
        const_pool = ctx.enter_context(tc.tile_pool(name="const", bufs=1))
        psum_pool = ctx.enter_context(tc.tile_pool(name="psum", bufs=2, space="PSUM"))
        work_pool = ctx.enter_context(tc.tile_pool(name="work", bufs=3))

        lr = const_pool.tile([KSPL, HALF], F32R, tag="lr")
        nc.gpsimd.dma_start(lr[:], lhs_row[:])
        rr = const_pool.tile([KSPL, M], F32R, tag="rr")
        nc.gpsimd.dma_start(rr[:], rhs_row[:])
        lc = const_pool.tile([KSPL, HALF], F32R, tag="lc")
        nc.gpsimd.dma_start(lc[:], lhs_col[:])
        rc = const_pool.tile([KSPL, N], F32R, tag="rc")
        nc.gpsimd.dma_start(rc[:], rhs_col[:])
        ar = const_pool.tile([128, NBLK], F32, tag="ar")
        nc.gpsimd.dma_start(ar[:], add_row[:])
        ac = const_pool.tile([128, NBLK], F32, tag="ac")
        nc.gpsimd.dma_start(ac[:], add_col[:])

        def body(_iv=None):
            sums = work_pool.tile([128, 2], F32, tag="sums")
            for d, (lhs_s, rhs_s, add_s) in enumerate(
                [(lr, rr, ar), (lc, rc, ac)]
            ):
                mins_all = work_pool.tile([128, NBLK], F32, tag="mins_all")
                for i in range(NBLK):
                    lslice = lhs_s[:, i * 128:(i + 1) * 128]
                    minpart = work_pool.tile([128, NK], F32, tag="minpart")
                    for k in range(NK):
                        pk = psum_pool.tile([128, W], F32, tag="pt")
                        for q in range(W // MTILE):
                            j = (W // MTILE) * k + q
                            nc.tensor.matmul(
                                out=pk[:, q * MTILE:(q + 1) * MTILE],
                                lhsT=lslice,
                                rhs=rhs_s[:, j * MTILE:(j + 1) * MTILE],
                                start=True, stop=True,
                            )
                        nc.vector.tensor_reduce(
                            out=minpart[:, k:k + 1], in_=pk[:],
                            axis=mybir.AxisListType.X, op=mybir.AluOpType.min,
                        )
                    nc.vector.tensor_reduce(
                        out=mins_all[:, i:i + 1], in_=minpart[:],
                        axis=mybir.AxisListType.X, op=mybir.AluOpType.min,
                    )
                sq = work_pool.tile([128, NBLK], F32, tag="sq")
                nc.vector.tensor_tensor(
                    out=sq[:], in0=mins_all[:], in1=add_s[:],
                    op=mybir.AluOpType.add,
                )
                nc.scalar.activation(
                    out=sq[:], in_=sq[:], func=mybir.ActivationFunctionType.Relu,
                )
                nc.scalar.activation(
                    out=sq[:], in_=sq[:], func=mybir.ActivationFunctionType.Sqrt,
                )
                nc.vector.tensor_reduce(
                    out=sums[:, d:d + 1], in_=sq[:],
                    axis=mybir.AxisListType.X, op=mybir.AluOpType.add,
                )
            nc.gpsimd.dma_start(out[:], sums[:])

        # static unroll for timing builds (tc.For_i loop machinery emits
        # raw-ISA branch instructions this walrus build rejects)
        for _ in range(reps):
            body()

    _split_excess_waits(nc)
    return nc


def _mask_f32r(x):
    b = np.asarray(x, dtype=np.float32).view(np.uint32) & np.uint32(0xFFFFF000)
    return b.view(np.float32)


def _split2(x):
    t1 = _mask_f32r(x)
    t2 = (np.asarray(x, dtype=np.float32) - t1).astype(np.float32)
    return t1, t2


def _prep_core_inputs_v2(template, source, core):
    """v2 packing: exact 2-term f32r split stacked into K.

    Row-dir E[n,m] = sum_d (-2 a_d[n]) b_d[m] + b2[m]; each factor split
    x = x1 + x2 with x1 = mask(x) (top 11 mantissa bits), x2 = x - x1;
    all 4 cross products per dim kept -> products exact in fp32 PSUM.
    K layout (KSPL=14): per dim d: rows (u1,v1),(u1,v2),(u2,v1),(u2,v2);
    then (1,w1),(1,w2) for the b2 term."""
    b, h = core // 2, core % 2
    a = np.ascontiguousarray(template[b], dtype=np.float32)   # [N, 3]
    s = np.ascontiguousarray(source[b], dtype=np.float32)     # [M, 3]
    a2 = np.sum(a * a, axis=1, dtype=np.float32)
    s2 = np.sum(s * s, axis=1, dtype=np.float32)
    lo, hi = h * HALF, (h + 1) * HALF

    def pack(xch, yfull, y2full):
        # lhs rows from -2*x chunk (split), rhs rows from y full (split)
        nrow = xch.shape[0]
        lhs = np.empty((KSPL, nrow), np.float32)
        rhs = np.empty((KSPL, yfull.shape[0]), np.float32)
        k = 0
        for d_ in range(D):
            x1, x2 = _split2(-2.0 * xch[:, d_])
            y1, y2 = _split2(yfull[:, d_])
            for u, v in ((x1, y1), (x1, y2), (x2, y1), (x2, y2)):
                lhs[k] = u
                rhs[k] = v
                k += 1
        w1, w2 = _split2(y2full)
        for w in (w1, w2):
            lhs[k] = 1.0
            rhs[k] = w
            k += 1
        assert k == KSPL
        return lhs, rhs

    lhs_row, rhs_row = pack(a[lo:hi], s, s2)
    lhs_col, rhs_col = pack(s[lo:hi], a, a2)

    def addmat(x2_chunk):
        return np.ascontiguousarray(
            x2_chunk.reshape(NBLK, 128).T, dtype=np.float32
        )

    return {
        "lhs_row": lhs_row, "rhs_row": rhs_row,
        "lhs_col": lhs_col, "rhs_col": rhs_col,
        "add_row": addmat(a2[lo:hi]),
        "add_col": addmat(s2[lo:hi]),
    }


def _prep_core_inputs(template, source, core):
    if MODE == "v2":
        return _prep_core_inputs_v2(template, source, core)
    return _prep_core_inputs_v1(template, source, core)


def _prep_core_inputs_v1(template, source, core):
    """Host-side packing for one core: batch b=core//2, half h=core%2."""
    b, h = core // 2, core % 2
    a = np.ascontiguousarray(template[b], dtype=np.float32)   # [N, 3]
    s = np.ascontiguousarray(source[b], dtype=np.float32)     # [M, 3]
    a2 = np.sum(a * a, axis=1)
    s2 = np.sum(s * s, axis=1)
    lo, hi = h * HALF, (h + 1) * HALF
    ach, s_ch = a[lo:hi], s[lo:hi]

    def aug_lhs(x):
        # [-2x_d rows; ones] -> [KAUG, HALF]
        return np.concatenate(
            [-2.0 * x.T, np.ones((1, x.shape[0]), np.float32)], axis=0
        ).astype(np.float32)

    def aug_rhs(x, x2):
        # [x_d rows; x2] -> [KAUG, N]
        return np.concatenate([x.T, x2[None, :]], axis=0).astype(np.float32)

    def addmat(x2_chunk):
        # a2[n = i*128 + p] at [p, i]
        return np.ascontiguousarray(
            x2_chunk.reshape(NBLK, 128).T, dtype=np.float32
        )

    return {
        "lhs_row": aug_lhs(ach),
        "rhs_row": aug_rhs(s, s2),
        "lhs_col": aug_lhs(s_ch),
        "rhs_col": aug_rhs(a, a2),
        "add_row": addmat(a2[lo:hi]),
        "add_col": addmat(s2[lo:hi]),
    }


_cached = {}


def _get_kernel(reps=1):
    if reps not in _cached:
        _cached[reps] = build_kernel(reps)
    return _cached[reps]


def run_on_hw(template, source, reps=1):
    nc = _get_kernel(reps)
    in_maps = [_prep_core_inputs(template, source, c) for c in range(N_CORES)]
    res = run_bass_kernel_spmd(nc, in_maps, list(range(N_CORES)))
    return res


def kernel(template, source):
    template = np.asarray(template, dtype=np.float32)
    source = np.asarray(source, dtype=np.float32)
    res = run_on_hw(template, source)
    row_total = 0.0
    col_total = 0.0
    for c in range(N_CORES):
        o = np.asarray(res.results[c]["out"], dtype=np.float64)
        row_total += o[:, 0].sum()
        col_total += o[:, 1].sum()
    # row-dir = min over sources for each template point -> mean over B*N
    # col-dir = min over templates for each source point -> mean over B*M
    val = col_total / (B * M) + row_total / (B * N)
    return np.array(val, dtype=np.float32)


if __name__ == "__main__":
    rng = np.random.default_rng(0)
    t = rng.standard_normal((B, N, D)).astype(np.float32)
    s = rng.standard_normal((B, M, D)).astype(np.float32)
    got = kernel(t, s)
    # numpy reference
    import numpy.linalg as la
    tot = 0.0
    mins_n = []
    mins_m = []
    for b in range(B):
        d2 = (
            (t[b] ** 2).sum(1)[:, None]
            + (s[b] ** 2).sum(1)[None, :]
            - 2.0 * t[b] @ s[b].T
        )
        d = np.sqrt(np.maximum(d2, 0))
        mins_m.append(d.min(axis=1))
        mins_n.append(d.min(axis=0))
    exp = np.concatenate(mins_n).mean() + np.concatenate(mins_m).mean()
    print("got", got, "exp", exp, "rel", abs(got - exp) / abs(exp))
